# revision 37
# baseline (speedup 1.0000x reference)
"""Contrastive distance loss (CLIP-style with squared-Euclidean logits) on 8 TRN2 cores.

Math:
  logits[i,j] = -||t_i - p_j||^2 / TEMP = S*(cross_ij - tsq_i/2 - psq_j/2),  S = 2/TEMP
  loss = 0.5*(mean_i(lse_row_i - diag_i) + mean_j(lse_col_j - diag_j))

Sharding: rows of `target` are split across 8 cores (data parallel); every core
holds the full `prediction` (the "all-gather" is free because the host hands the
kernel full inputs). Each core computes its 1024x8192 block of the logits and
reduces it to row-wise (max, sumexp) partials per 512-column chunk and
column-wise (max, sumexp) partials over its local 1024 rows. The host combines
the tiny partials in float64 (standard streaming-logsumexp merge) - the
distributed all-reduce of the two CE sums collapses to this gather.

Device pipeline per core:
  - t/p tiles are cast to bf16 in-flight (SWDGE cast-DMA, one large DMA per
    chunk) and transposed to [d, i] / [d, j] layout with the DMA xbar transpose.
  - GEMM in bf16 on the PE with fp32 PSUM accumulation. An extra K=4
    contraction block folds -tsq/2 (hi/lo bf16 split) and -psq/2 into the same
    accumulation, so PSUM holds -d^2/2 directly.
  - row path fused into ONE VectorE op via the TENSOR_MASK_REDUCE custom DVE
    op: L = PSUM * S (+logits) with rowmax accumulated in the same pass; a
    tiny negate produces the exp bias; exp+sum in one ScalarE activation
    (per-partition bias, accum_out).
  - column stats: PE transposes L 128x128 blocks into PSUM, ScalarE/VectorE
    copy them into a column-major strip, then one max-reduce + negate +
    exp+accum per 128-column block covers all 1024 local rows.
  - diag_i = -||t_i - p_i||^2/TEMP from the bf16 inputs (exact diag would need
    an extra fp32 stream; bf16 keeps the final rel-err at ~2e-5).
  - chunk prep (cast-DMA, psq, nps extras, xbar transposes) is software-
    pipelined one chunk ahead of the GEMM consuming it.
"""

import numpy as np
from contextlib import ExitStack

import concourse.bacc as bacc
import concourse.tile as tile
import concourse.mybir as mybir
from concourse import bass_utils, masks
from concourse.dve_ops import TENSOR_MASK_REDUCE

F32 = mybir.dt.float32
BF16 = mybir.dt.bfloat16

N, D = 8192, 1024
TEMP = 0.07
S = 2.0 / TEMP
NCORES = 8
NLOC = N // NCORES          # 1024 rows of target per core
MT = NLOC // 128            # 8 m-tiles
KC = D // 128               # 8 contraction chunks
NJ = 512                    # output-tile width (one PSUM bank, fp32)
NCHUNK = N // NJ            # 16 column chunks
JB = NJ // 128              # 4 j-blocks per chunk

_prog_cache = None


def _build_program():
    nc = bacc.Bacc("TRN2", target_bir_lowering=False, debug=False)

    t_d = nc.dram_tensor("t_loc", [NLOC, D], F32, kind="ExternalInput").ap()
    p_d = nc.dram_tensor("p_full", [N, D], F32, kind="ExternalInput").ap()
    ploc_d = nc.dram_tensor("p_loc", [NLOC, D], F32, kind="ExternalInput").ap()

    rnm_d = nc.dram_tensor("row_negmax", [128, MT, NCHUNK], F32, kind="ExternalOutput").ap()
    rse_d = nc.dram_tensor("row_sumexp", [128, MT, NCHUNK], F32, kind="ExternalOutput").ap()
    cnm_d = nc.dram_tensor("col_negmax", [128, NCHUNK, JB], F32, kind="ExternalOutput").ap()
    cse_d = nc.dram_tensor("col_sumexp", [128, NCHUNK, JB], F32, kind="ExternalOutput").ap()
    diag_d = nc.dram_tensor("diag", [128, MT], F32, kind="ExternalOutput").ap()

    AF = mybir.ActivationFunctionType
    OP = mybir.AluOpType
    AX = mybir.AxisListType

    with tile.TileContext(nc) as tc, ExitStack() as ctx:
        persist = ctx.enter_context(tc.tile_pool(name="persist", bufs=1))
        psum_small = ctx.enter_context(tc.tile_pool(name="psum_small", bufs=1, space="PSUM"))
        pchunk = ctx.enter_context(tc.tile_pool(name="pchunk", bufs=2))
        work = ctx.enter_context(tc.tile_pool(name="work", bufs=3))
        lpool = ctx.enter_context(tc.tile_pool(name="lpool", bufs=4))
        psum_l_pool = ctx.enter_context(tc.tile_pool(name="psum_l", bufs=4, space="PSUM"))
        psum_t_pool = ctx.enter_context(tc.tile_pool(name="psum_t", bufs=3, space="PSUM"))

        ident = persist.tile([128, 128], F32)
        masks.make_identity(nc, ident[:])
        mend512 = persist.tile([128, 1], F32)
        nc.vector.memset(mend512[:], float(NJ))

        # persistent operand / stats tiles
        ttb = persist.tile([128, KC, NLOC], BF16)          # t^T in [d, k, i] layout
        extras_lhsT = persist.tile([4, MT, 128], BF16)     # [nts_hi; nts_lo; 1; 1] per m
        rstats_nm = persist.tile([128, MT, NCHUNK], F32)
        rstats_se = persist.tile([128, MT, NCHUNK], F32)
        cstats_nm = persist.tile([128, NCHUNK, JB], F32)
        cstats_se = persist.tile([128, NCHUNK, JB], F32)
        diag_sb = persist.tile([128, MT], F32)
        ssum = persist.tile([128, MT], F32)                # sum (t-p)^2 per row
        tsqc = persist.tile([128, MT], F32)                # sum t^2 per row

        def prep_chunk(n):
            """cast-DMA the chunk, compute psq -> nps extras rows, xbar-transpose."""
            j0 = n * NJ
            psq4 = work.tile([128, JB], F32, tag="psq4")
            pb4 = pchunk.tile([128, JB, D], BF16, tag="pb4")
            if n == 0:
                for s in range(JB):
                    nc.gpsimd.dma_start(out=pb4[:, s, :],
                                        in_=p_d[j0 + s * 128:j0 + (s + 1) * 128, :])
            else:
                nc.gpsimd.dma_start(
                    out=pb4[:],
                    in_=p_d[j0:j0 + NJ, :].rearrange("(s p) d -> p s d", p=128))
            for s in range(JB):
                sqp = work.tile([128, D], BF16, tag="sqp")
                if s % 2 == 0:
                    nc.scalar.activation(out=sqp[:], in_=pb4[:, s, :], func=AF.Square,
                                         accum_out=psq4[:, s:s + 1])
                else:
                    nc.vector.scalar_tensor_tensor(out=sqp[:], in0=pb4[:, s, :], scalar=1.0,
                                                   in1=pb4[:, s, :], op0=OP.mult, op1=OP.mult,
                                                   accum_out=psq4[:, s:s + 1])

            ptb = pchunk.tile([128, KC, NJ], BF16, tag="ptb")
            for s in range(JB):
                nc.sync.dma_start_transpose(ptb[:, :, s * 128:(s + 1) * 128], pb4[:, s, :])

            npsm = work.tile([128, JB], F32, tag="npsm")
            nc.vector.tensor_scalar_mul(npsm[:], psq4[:], -0.5)
            ps4 = psum_small.tile([JB, 128], F32, tag="pssmall")
            nc.tensor.transpose(ps4[:], npsm[:], ident[:])
            npsT = work.tile([JB, 128], F32, tag="npsT")
            nc.vector.tensor_copy(npsT[:], ps4[:])
            npsT_hi = work.tile([JB, 128], BF16, tag="npsT_hi")
            nc.vector.tensor_copy(npsT_hi[:], npsT[:])
            npsT_lo = work.tile([JB, 128], BF16, tag="npsT_lo")
            nc.vector.tensor_tensor(out=npsT_lo[:], in0=npsT[:], in1=npsT_hi[:], op=OP.subtract)

            extras_rhs = work.tile([4, NJ], BF16, tag="extras_rhs")
            nc.vector.memset(extras_rhs[0:4, :], 1.0)
            nc.sync.dma_start(out=extras_rhs[2:3, :], in_=npsT_hi[:, :])
            nc.sync.dma_start(out=extras_rhs[3:4, :], in_=npsT_lo[:, :])

            return extras_rhs, ptb

        # ---------- prime the pipeline: chunk 0 prep first ----------
        prepped = prep_chunk(0)

        # ---------- phase 0: target prep, diag, nts (all bf16) ----------
        with tc.tile_pool(name="prep", bufs=2) as prep:
            tball = prep.tile([128, MT, D], BF16, tag="tball", bufs=1)
            plball = prep.tile([128, MT, D], BF16, tag="plball", bufs=1)
            half = MT // 2
            nc.gpsimd.dma_start(out=tball[:, 0:half, :],
                                in_=t_d[0:half * 128, :].rearrange("(m p) d -> p m d", p=128))
            nc.gpsimd.dma_start(out=tball[:, half:MT, :],
                                in_=t_d[half * 128:, :].rearrange("(m p) d -> p m d", p=128))
            nc.gpsimd.dma_start(out=plball[:], in_=ploc_d.rearrange("(m p) d -> p m d", p=128))
            for m in range(MT):
                tb = tball[:, m, :]
                nc.sync.dma_start_transpose(ttb[:, :, m * 128:(m + 1) * 128], tb)

                dtmp = prep.tile([128, D], BF16, tag="dtmp")
                nc.vector.tensor_tensor(out=dtmp[:], in0=tb, in1=plball[:, m, :], op=OP.subtract)
                sq1 = prep.tile([128, D], BF16, tag="sq1")
                nc.scalar.activation(out=sq1[:], in_=dtmp[:], func=AF.Square,
                                     accum_out=ssum[:, m:m + 1])
                sq2 = prep.tile([128, D], BF16, tag="sq2")
                nc.scalar.activation(out=sq2[:], in_=tb, func=AF.Square,
                                     accum_out=tsqc[:, m:m + 1])

            nc.vector.tensor_scalar_mul(diag_sb[:], ssum[:], -1.0 / TEMP)

            # nts = -tsq/2 -> transpose to [m, i] rows -> bf16 hi/lo extras rows
            nts = prep.tile([128, MT], F32, tag="nts")
            nc.vector.tensor_scalar_mul(nts[:], tsqc[:], -0.5)
            ps8 = psum_small.tile([MT, 128], F32, tag="pssmall")
            nc.tensor.transpose(ps8[:], nts[:], ident[:])
            ntsT = prep.tile([MT, 128], F32, tag="ntsT")
            nc.vector.tensor_copy(ntsT[:], ps8[:])
            ntsT_hi = prep.tile([MT, 128], BF16, tag="ntsT_hi")
            nc.vector.tensor_copy(ntsT_hi[:], ntsT[:])
            ntsT_lo = prep.tile([MT, 128], BF16, tag="ntsT_lo")
            nc.vector.tensor_tensor(out=ntsT_lo[:], in0=ntsT[:], in1=ntsT_hi[:], op=OP.subtract)

            nc.gpsimd.memset(extras_lhsT[0:4, :, :], 1.0)
            nc.sync.dma_start(out=extras_lhsT[0:1, :, :], in_=ntsT_hi[:, :])
            nc.sync.dma_start(out=extras_lhsT[1:2, :, :], in_=ntsT_lo[:, :])

        # ---------- phase 1: main loop, prep pipelined one chunk ahead ----------
        for n in range(NCHUNK):
            extras_rhs, ptb = prepped
            prepped_next = prep_chunk(n + 1) if n + 1 < NCHUNK else None

            # column-major strip of -L^T for this chunk: [j_in_block, jb, m, i]
            lts = work.tile([128, JB, MT, 128], F32, tag="lts")

            for m in range(MT):
                psl = psum_l_pool.tile([128, NJ], F32, tag="psl")
                for k in range(KC):
                    nc.tensor.matmul(psl[:], ttb[:, k, m * 128:(m + 1) * 128], ptb[:, k, :],
                                     start=(k == 0), stop=False)
                nc.tensor.matmul(psl[:], extras_lhsT[:, m, :], extras_rhs[:],
                                 start=False, stop=True)

                lsb = lpool.tile([128, NJ], F32, tag="lsb")
                rmaxp = lpool.tile([128, 1], F32, tag="rmaxp")
                # fused: lsb = psl * S (= +logits), rmaxp = rowmax(lsb)
                nc.vector._custom_dve(TENSOR_MASK_REDUCE, out=lsb[:], in0=psl[:],
                                      in1=mend512[:], s0=0.0, s1=-3.0e38, imm2=S,
                                      accum_out=rmaxp[:])
                nc.vector.tensor_scalar_mul(rstats_nm[:, m, n:n + 1], rmaxp[:], -1.0)
                escr = lpool.tile([128, NJ], BF16, tag="escr")
                nc.scalar.activation(out=escr[:], in_=lsb[:], func=AF.Exp,
                                     bias=rstats_nm[:, m, n:n + 1], scale=1.0,
                                     accum_out=rstats_se[:, m, n:n + 1])

                pst = psum_t_pool.tile([128, NJ], F32, tag="pst")
                for b in range(JB):
                    nc.tensor.transpose(pst[:, b * 128:(b + 1) * 128],
                                        lsb[:, b * 128:(b + 1) * 128], ident[:])
                if m % 2 == 0:
                    nc.scalar.copy(out=lts[:, :, m, :], in_=pst[:].rearrange("p (b i) -> p b i", b=JB))
                else:
                    nc.vector.tensor_copy(lts[:, :, m, :], pst[:].rearrange("p (b i) -> p b i", b=JB))

            for b in range(JB):
                cmaxp = lpool.tile([128, 1], F32, tag="cmaxp")
                nc.vector.tensor_reduce(out=cmaxp[:], in_=lts[:, b, :, :],
                                        axis=AX.XY, op=OP.max)
                nc.vector.tensor_scalar_mul(cstats_nm[:, n, b:b + 1], cmaxp[:], -1.0)
                cescr = lpool.tile([128, MT * 128], BF16, tag="cescr")
                nc.scalar.activation(out=cescr[:].rearrange("p (m i) -> p m i", m=MT),
                                     in_=lts[:, b, :, :], func=AF.Exp,
                                     bias=cstats_nm[:, n, b:b + 1], scale=1.0,
                                     accum_out=cstats_se[:, n, b:b + 1])

            prepped = prepped_next

        # ---------- phase 2: write stats ----------
        nc.sync.dma_start(out=rnm_d[:], in_=rstats_nm[:])
        nc.sync.dma_start(out=rse_d[:], in_=rstats_se[:])
        nc.sync.dma_start(out=cnm_d[:], in_=cstats_nm[:])
        nc.sync.dma_start(out=cse_d[:], in_=cstats_se[:])
        nc.sync.dma_start(out=diag_d[:], in_=diag_sb[:])

    nc.compile()
    return nc


def _get_program():
    global _prog_cache
    if _prog_cache is None:
        _prog_cache = _build_program()
    return _prog_cache


def _run(prediction, target, trace=False):
    prediction = np.ascontiguousarray(np.asarray(prediction, dtype=np.float32))
    target = np.ascontiguousarray(np.asarray(target, dtype=np.float32))
    assert prediction.shape == (N, D) and target.shape == (N, D)

    nc = _get_program()
    in_maps = []
    for c in range(NCORES):
        rows = slice(c * NLOC, (c + 1) * NLOC)
        in_maps.append({
            "t_loc": target[rows],
            "p_full": prediction,
            "p_loc": prediction[rows],
        })
    res = bass_utils.run_bass_kernel_spmd(nc, in_maps, core_ids=list(range(NCORES)),
                                          trace=trace)

    # ---------- host combine (tiny, float64) ----------
    # global row index: i = c*1024 + m*128 + p  <->  per-core arrays [p, m, ...]
    row_max = np.empty((N, NCHUNK))
    row_se = np.empty((N, NCHUNK))
    diag = np.empty(N)
    col_max_c = np.empty((NCORES, N))
    col_se_c = np.empty((NCORES, N))
    for c, r in enumerate(res.results):
        rm = -r["row_negmax"].astype(np.float64)     # [128, MT, NCHUNK]
        rs = r["row_sumexp"].astype(np.float64)
        dg = r["diag"].astype(np.float64)            # [128, MT]
        row_max[c * NLOC:(c + 1) * NLOC] = rm.transpose(1, 0, 2).reshape(NLOC, NCHUNK)
        row_se[c * NLOC:(c + 1) * NLOC] = rs.transpose(1, 0, 2).reshape(NLOC, NCHUNK)
        diag[c * NLOC:(c + 1) * NLOC] = dg.T.reshape(NLOC)
        cm = -r["col_negmax"].astype(np.float64)     # [128, NCHUNK, JB], j = n*512 + b*128 + p
        cs = r["col_sumexp"].astype(np.float64)
        col_max_c[c] = cm.transpose(1, 2, 0).reshape(N)
        col_se_c[c] = cs.transpose(1, 2, 0).reshape(N)

    M_r = row_max.max(axis=1)
    lse_row = M_r + np.log((row_se * np.exp(row_max - M_r[:, None])).sum(axis=1))
    M_c = col_max_c.max(axis=0)
    lse_col = M_c + np.log((col_se_c * np.exp(col_max_c - M_c[None, :])).sum(axis=0))

    ce_rows = (lse_row - diag).mean()
    ce_cols = (lse_col - diag).mean()
    out = np.float32((ce_rows + ce_cols) * 0.5)
    return out, res


def kernel(prediction, target):
    out, _ = _run(prediction, target, trace=False)
    return out
